# revision 3
# baseline (speedup 1.0000x reference)
"""Trainium2 Bass kernel for nn_ClassifyModelMOE (conv feature extractor +
top-3-of-5 MoE + softmax head). Data-parallel over batch across 8 cores.

Self-contained: hardcodes all shapes; builds Toeplitz-expanded conv weights on
the host; runs one SPMD Bass/Tile program on cores 0-7 via run_bass_kernel_spmd.

v2: host-transposed x (plain strided DMAs, no DMA-transpose), SBUF-resident
expert weights (1 startup DMA instead of 250/rep), pool-before-relu in conv1,
batched gating/head per chunk, merged output DMA.
"""
import os
import sys

sys.path.insert(0, "/opt/trn_rl_repo")

import numpy as np
import ml_dtypes

import concourse.bacc as bacc
import concourse.mybir as mybir
import concourse.tile as tile
from concourse.bass_utils import run_bass_kernel_spmd
from concourse.masks import make_identity

F32 = mybir.dt.float32
BF16 = mybir.dt.bfloat16
AF = mybir.ActivationFunctionType
ALU = mybir.AluOpType
AX = mybir.AxisListType

NCORES = 8
B = 8192
BC = B // NCORES          # tokens per core
NB = 512                  # batch chunk (columns per matmul)
NCH = BC // NB            # chunks per core
NE, TOPK = 5, 3
DH = 128
NT4 = NB // 128           # 128-token groups per chunk

# conv1 output geometry: 16ch x 24x24; M-layout (per output row y):
#   Mc0: even x = 2j, j=0..8   -> m = j*16 + o        (128)
#   Mc1: [even j=8..12 | odd j=8..12] -> 64+64        (128)
#   Mc2: odd x = 2j+1, j=0..8  -> m = j*16 + o        (128)
# pooled row tiles: PP0 = j 0..8 (128 parts: j*16+c), PP1 = j 8..12 (64 parts)
# conv2 output (per row y): M = xout*32 + o2:
#   Mc0: xout 0..4 (128), Mc1: xout 4..8 (128),
#   Mc2pair: [y0: xout 8..10 | y1: xout 8..10] (64+64)


def _conv1_cols():
    """(x, o) per (mc, col) for the conv1 M-layout."""
    cols = {0: [], 1: [], 2: []}
    for j in range(8):
        for o in range(16):
            cols[0].append((2 * j, o))
            cols[2].append((2 * j + 1, o))
    for j in range(8, 12):
        for o in range(16):
            cols[1].append((2 * j, o))
    for j in range(8, 12):
        for o in range(16):
            cols[1].append((2 * j + 1, o))
    return cols


def _host_prep(x, conv1_w, conv1_b, conv2_w, conv2_b, gate_w, gate_b,
               e1_w, e1_b, e2_w, e2_b, sm_w, sm_b):
    x = np.asarray(x, np.float32)
    conv1_w = np.asarray(conv1_w, np.float32)
    conv2_w = np.asarray(conv2_w, np.float32)
    gate_w = np.asarray(gate_w, np.float32)
    e1_w = np.asarray(e1_w, np.float32)
    e2_w = np.asarray(e2_w, np.float32)
    bf = ml_dtypes.bfloat16

    # x transposed on host: row 32*r + c = pixel (r, c), col = token.
    xpT = np.zeros((1024, B), np.float32)
    xr = x.reshape(B, 28, 28)
    for r in range(28):
        xpT[32 * r:32 * r + 28, :] = xr[:, r, :].T
    xpT = xpT.astype(bf)

    cols = _conv1_cols()
    # w1c1 [128, 3*128]: K-row = 32*s + ci (s=0..3 -> dy), cols per Mc
    w1c1 = np.zeros((128, 384), np.float32)
    w1c2 = np.zeros((28, 384), np.float32)
    for mc in range(3):
        for ci_col, (xx, o) in enumerate(cols[mc]):
            col = 128 * mc + ci_col
            for s in range(4):
                for dx in range(5):
                    ci = xx + dx
                    w1c1[32 * s + ci, col] = conv1_w[o, 0, s, dx]
            for dx in range(5):
                w1c2[xx + dx, col] = conv1_w[o, 0, 4, dx]
    w1c1 = w1c1.astype(bf)
    w1c2 = w1c2.astype(bf)

    # conv2 toeplitz: pooled row layout p = j*16 + c (PP0: j<8), (j-8)*16+c (PP1)
    # w2p0 [3, 128, 256]: r taps, cols [Mc0 | Mc1]
    w2p0 = np.zeros((3, 128, 256), np.float32)
    w2p1mc1 = np.zeros((3, 64, 128), np.float32)
    w2p1mc2 = np.zeros((4, 64, 128), np.float32)
    for r in range(3):
        for j in range(8):
            for c in range(16):
                p = j * 16 + c
                for mci, xobase in ((0, 0), (1, 4)):
                    for xo in range(xobase, xobase + 4):
                        dx = j - xo
                        if 0 <= dx < 3:
                            for o2 in range(32):
                                w2p0[r, p, 128 * mci + (xo - xobase) * 32 + o2] = \
                                    conv2_w[o2, c, r, dx]
        for j in range(8, 12):
            for c in range(16):
                p = (j - 8) * 16 + c
                for xo in range(4, 8):
                    dx = j - xo
                    if 0 <= dx < 3:
                        for o2 in range(32):
                            w2p1mc1[r, p, (xo - 4) * 32 + o2] = conv2_w[o2, c, r, dx]
    for rr in range(4):
        for b_ in range(2):
            r = rr - b_
            if not (0 <= r < 3):
                continue
            for j in range(8, 12):
                for c in range(16):
                    p = (j - 8) * 16 + c
                    for xo in range(8, 10):
                        dx = j - xo
                        if 0 <= dx < 3:
                            for o2 in range(32):
                                w2p1mc2[rr, p, 64 * b_ + (xo - 8) * 32 + o2] = \
                                    conv2_w[o2, c, r, dx]

    # h feature permutation: our flat index (tile*128+part) -> reference f = o2*100 + y*10 + xo
    perm = np.zeros(3200, np.int64)
    for P in range(5):
        y0, y1 = 2 * P, 2 * P + 1
        tiles = []
        for yy in (y0, y1):
            for xobase in (0, 4):
                tiles.append([(yy, xo, o2) for xo in range(xobase, xobase + 4)
                              for o2 in range(32)])
        t4 = [(y0, xo, o2) for xo in range(8, 10) for o2 in range(32)] + \
             [(y1, xo, o2) for xo in range(8, 10) for o2 in range(32)]
        order = [tiles[0], tiles[1], tiles[2], tiles[3], t4]
        for ti, tl in enumerate(order):
            for p, (yy, xo, o2) in enumerate(tl):
                perm[(5 * P + ti) * 128 + p] = o2 * 100 + yy * 10 + xo
    # resident expert-1 weights: [K=128, (e,kc,m)] columns
    e1r = np.ascontiguousarray(
        e1_w[:, perm, :].reshape(NE, 25, 128, DH).transpose(2, 0, 1, 3)
    ).reshape(128, NE * 25 * DH)
    gwp = gate_w[perm, :].reshape(25, 128, NE).astype(np.float32)

    b1col = np.asarray(conv1_b, np.float32)[np.arange(128) % 16].reshape(128, 1)
    b2col = np.asarray(conv2_b, np.float32)[np.arange(128) % 32].reshape(128, 1)
    gbcol = np.asarray(gate_b, np.float32).reshape(NE, 1)
    e1bT = np.asarray(e1_b, np.float32).T.copy()      # [128, 5]
    e2bT = np.asarray(e2_b, np.float32).T.copy()      # [128, 5]
    smw = np.asarray(sm_w, np.float32)                # [128, 10]
    smb = np.tile(np.asarray(sm_b, np.float32), NE * NT4).reshape(1, NE * NT4 * 10)

    weights = dict(
        w1c1=w1c1, w1c2=w1c2,
        w2p0=np.ascontiguousarray(w2p0.transpose(1, 0, 2)).reshape(128, 768).astype(bf),
        w2p1mc1=np.ascontiguousarray(w2p1mc1.transpose(1, 0, 2)).reshape(64, 384).astype(bf),
        w2p1mc2=np.ascontiguousarray(w2p1mc2.transpose(1, 0, 2)).reshape(64, 512).astype(bf),
        e1r=e1r.astype(bf),
        gwp=np.ascontiguousarray(gwp.transpose(1, 0, 2)).reshape(128, 125).astype(bf),
        e2w=np.ascontiguousarray(e2_w.astype(np.float32).transpose(1, 0, 2)).reshape(128, 640).astype(bf),
        b1col=b1col, b2col=b2col,
        gbcol=gbcol, e1bT=e1bT, e2bT=e2bT, smw=smw.astype(bf), smb=smb.astype(bf))
    return xpT, weights


def _build_nc(loop_reps=None):
    nc = bacc.Bacc("TRN2", target_bir_lowering=False, debug=False)
    d = {}
    def din(name, shape, dt):
        d[name] = nc.dram_tensor(name, list(shape), dt, kind="ExternalInput").ap()
    din("xpT", (1024, BC), BF16)
    din("w1c1", (128, 384), BF16)
    din("w1c2", (28, 384), BF16)
    din("w2p0", (128, 768), BF16)
    din("w2p1mc1", (64, 384), BF16)
    din("w2p1mc2", (64, 512), BF16)
    din("e1r", (128, NE * 25 * DH), BF16)
    din("gwp", (128, 125), BF16)
    din("e2w", (128, 640), BF16)
    din("b1col", (128, 1), F32)
    din("b2col", (128, 1), F32)
    din("gbcol", (NE, 1), F32)
    din("e1bT", (128, NE), F32)
    din("e2bT", (128, NE), F32)
    din("smw", (128, 10), BF16)
    din("smb", (1, NE * NT4 * 10), BF16)
    out_d = nc.dram_tensor("out", [BC, 10], F32, kind="ExternalOutput").ap()

    with tile.TileContext(nc) as tc:
        _emit(nc, tc, d, out_d, loop_reps=loop_reps)
    nc.compile()
    return nc


def _emit(nc, tc, d, out_d, loop_reps=None):
    import contextlib
    ctx = contextlib.ExitStack()
    with ctx:
        wpool = ctx.enter_context(tc.tile_pool(name="wpool", bufs=1))
        xtp = ctx.enter_context(tc.tile_pool(name="xtp", bufs=2))
        tmp = ctx.enter_context(tc.tile_pool(name="tmp", bufs=6))
        prp = ctx.enter_context(tc.tile_pool(name="prp", bufs=4))
        shp = ctx.enter_context(tc.tile_pool(name="shp", bufs=2))
        pp0p = ctx.enter_context(tc.tile_pool(name="pp0p", bufs=7))
        pp1p = ctx.enter_context(tc.tile_pool(name="pp1p", bufs=7))
        hpool = ctx.enter_context(tc.tile_pool(name="hpool", bufs=25))
        h1p = ctx.enter_context(tc.tile_pool(name="h1p", bufs=2))
        h2p = ctx.enter_context(tc.tile_pool(name="h2p", bufs=5))
        gp = ctx.enter_context(tc.tile_pool(name="gp", bufs=2))
        smallp = ctx.enter_context(tc.tile_pool(name="smallp", bufs=16))
        c1ps = ctx.enter_context(tc.tile_pool(name="c1ps", bufs=3, space="PSUM"))
        c2ps = ctx.enter_context(tc.tile_pool(name="c2ps", bufs=2, space="PSUM"))
        exps = ctx.enter_context(tc.tile_pool(name="exps", bufs=2, space="PSUM"))
        hdps = ctx.enter_context(tc.tile_pool(name="hdps", bufs=1, space="PSUM"))

        # resident weights
        w1c1 = wpool.tile([128, 384], BF16); nc.sync.dma_start(w1c1[:], d["w1c1"][:])
        w1c2 = wpool.tile([28, 384], BF16); nc.sync.dma_start(w1c2[:], d["w1c2"][:])
        w2p0 = wpool.tile([128, 3 * 256], BF16)
        nc.sync.dma_start(w2p0[:], d["w2p0"][:])
        w2p1a = wpool.tile([64, 3 * 128], BF16)
        nc.sync.dma_start(w2p1a[:], d["w2p1mc1"][:])
        w2p1b = wpool.tile([64, 4 * 128], BF16)
        nc.sync.dma_start(w2p1b[:], d["w2p1mc2"][:])
        gw = wpool.tile([128, 25 * NE], BF16)
        nc.sync.dma_start(gw[:], d["gwp"][:])
        e2w = wpool.tile([128, NE * DH], BF16)
        nc.sync.dma_start(e2w[:], d["e2w"][:])
        e1w = wpool.tile([128, NE * 25 * DH], BF16)
        for e in range(NE):
            sl = slice(e * 25 * DH, (e + 1) * 25 * DH)
            nc.sync.dma_start(e1w[:, sl], d["e1r"][:, sl])
        b1c = wpool.tile([128, 1], F32); nc.sync.dma_start(b1c[:], d["b1col"][:])
        b2c = wpool.tile([128, 1], F32); nc.sync.dma_start(b2c[:], d["b2col"][:])
        gbc = wpool.tile([NE, 1], F32); nc.sync.dma_start(gbc[:], d["gbcol"][:])
        e1bT = wpool.tile([128, NE], F32); nc.sync.dma_start(e1bT[:], d["e1bT"][:])
        e2bT = wpool.tile([128, NE], F32); nc.sync.dma_start(e2bT[:], d["e2bT"][:])
        smw = wpool.tile([128, 10], BF16); nc.sync.dma_start(smw[:], d["smw"][:])
        smb = wpool.tile([1, NE * NT4 * 10], BF16)
        nc.sync.dma_start(smb[:], d["smb"][:])
        ident = wpool.tile([128, 128], F32)
        make_identity(nc, ident[:])
        ones = wpool.tile([1, 128], BF16)
        nc.scalar.activation(ones[:], e2w[0:1, 0:128], AF.Copy, scale=0.0, bias=1.0)

        import contextlib as _ctl
        loop_cm = tc.For_i(0, loop_reps, 1) if loop_reps else _ctl.nullcontext()
        with loop_cm:
         for ch in range(NCH):
            b0 = ch * NB
            # ---- x^T tiles: 4 strided DMAs (k mod 4 groups, non-overlapping) ----
            xbig = xtp.tile([128, 28 * NB], BF16, tag="xT")
            xv = xbig[:].rearrange("p (gi g t) -> p g gi t", gi=7, g=4, t=NB)
            for g in range(4):
                src = d["xpT"][32 * g:32 * g + 896, b0:b0 + NB]
                nc.sync.dma_start(xv[:, g], src.rearrange("(k p) t -> p k t", p=128))

            def xT(k):
                return xbig[:, k * NB:(k + 1) * NB]

            # ---- conv1 (+bias+relu after max-pool) ----
            pp0, pp1 = [], []
            for Y in range(12):
                y0, y1 = 2 * Y, 2 * Y + 1
                tm = []
                for mc in range(3):
                    pss = []
                    for yy in (y0, y1):
                        ps = c1ps.tile([128, NB], F32, tag="ps")
                        nc.tensor.matmul(ps[:], w1c1[:, 128 * mc:128 * mc + 128],
                                         xT(yy), start=True, stop=False)
                        nc.tensor.matmul(ps[:], w1c2[:, 128 * mc:128 * mc + 128],
                                         xT(yy + 4)[0:28, :], start=False, stop=True)
                        pss.append(ps)
                    # PSUM-PSUM tensor ops are illegal: stage y0 through SBUF
                    c = tmp.tile([128, NB], BF16, tag="cp")
                    nc.scalar.activation(c[:], pss[0][:], AF.Copy)
                    t = tmp.tile([128, NB], BF16, tag="tm")
                    nc.vector.tensor_tensor(t[:], pss[1][:], c[:], op=ALU.max)
                    tm.append(t)
                pre0 = prp.tile([128, NB], BF16, tag="pre0")
                nc.vector.tensor_tensor(pre0[:], tm[0][:], tm[2][:], op=ALU.max)
                p0 = pp0p.tile([128, NB], BF16, tag="pp0")
                nc.scalar.activation(p0[:], pre0[:], AF.Relu, bias=b1c[:, 0:1])
                sh = shp.tile([64, NB], BF16, tag="sh")
                nc.sync.dma_start(sh[:], tm[1][64:128, :])
                pre1 = prp.tile([64, NB], BF16, tag="pre1")
                nc.vector.tensor_tensor(pre1[:], tm[1][0:64, :], sh[:], op=ALU.max)
                p1 = pp1p.tile([64, NB], BF16, tag="pp1")
                nc.scalar.activation(p1[:], pre1[:], AF.Relu, bias=b1c[0:64, 0:1])
                pp0.append(p0)
                pp1.append(p1)

            # ---- conv2 + relu -> h tiles ----
            htiles = []
            for P in range(5):
                y0 = 2 * P
                for yy in (y0, y0 + 1):
                    for mci in range(2):
                        ps = c2ps.tile([128, NB], F32, tag="ps")
                        for r in range(3):
                            nc.tensor.matmul(
                                ps[:], w2p0[:, 256 * r + 128 * mci:256 * r + 128 * mci + 128],
                                pp0[yy + r][:], start=(r == 0),
                                stop=(mci == 0 and r == 2))
                        if mci == 1:
                            for r in range(3):
                                nc.tensor.matmul(ps[:], w2p1a[:, 128 * r:128 * r + 128],
                                                 pp1[yy + r][:], start=False,
                                                 stop=(r == 2))
                        h = hpool.tile([128, NB], BF16, tag="h")
                        nc.scalar.activation(h[:], ps[:], AF.Relu, bias=b2c[:, 0:1])
                        htiles.append(h)
                ps = c2ps.tile([128, NB], F32, tag="ps")
                for rr in range(4):
                    nc.tensor.matmul(ps[:], w2p1b[:, 128 * rr:128 * rr + 128],
                                     pp1[y0 + rr][:], start=(rr == 0), stop=(rr == 3))
                h = hpool.tile([128, NB], BF16, tag="h")
                nc.scalar.activation(h[:], ps[:], AF.Relu, bias=b2c[:, 0:1])
                htiles.append(h)

            # ---- gate ----
            gps = hdps.tile([NE, NB], F32, tag="hd")
            for kc in range(25):
                nc.tensor.matmul(gps[:], gw[:, NE * kc:NE * kc + NE], htiles[kc][:],
                                 start=(kc == 0), stop=(kc == 24))
            gsb = gp.tile([NE, NB], F32, tag="gsb")
            nc.scalar.activation(gsb[:], gps[:], AF.Identity, bias=gbc[:, 0:1])

            # ---- experts ----
            h2t = []
            for e in range(NE):
                h1ps = exps.tile([128, NB], F32, tag="exps")
                for kc in range(25):
                    nc.tensor.matmul(
                        h1ps[:], e1w[:, (e * 25 + kc) * DH:(e * 25 + kc + 1) * DH],
                        htiles[kc][:], start=(kc == 0), stop=(kc == 24))
                h1 = h1p.tile([128, NB], BF16, tag="h1")
                nc.scalar.activation(h1[:], h1ps[:], AF.Tanh, bias=e1bT[:, e:e + 1])
                h2ps = exps.tile([128, NB], F32, tag="exps")
                nc.tensor.matmul(h2ps[:], e2w[:, DH * e:DH * e + DH], h1[:],
                                 start=True, stop=True)
                h2 = h2p.tile([128, NB], BF16, tag="h2")
                nc.scalar.activation(h2[:], h2ps[:], AF.Tanh, bias=e2bT[:, e:e + 1])
                h2t.append(h2)

            # ---- batched gating weights (all 4 t4 groups at once) ----
            gtp = hdps.tile([128, NT4 * NE], F32, tag="hd")
            gtv = gtp[:].rearrange("p (t e) -> p t e", e=NE)
            for t4 in range(NT4):
                tok = slice(128 * t4, 128 * t4 + 128)
                nc.tensor.transpose(gtv[:, t4], gsb[:, tok], ident[0:NE, 0:NE])
            mx = smallp.tile([128, NT4], F32, tag="mx")
            nc.vector.reduce_max(mx[:], gtv, axis=AX.X)
            s = smallp.tile([128, NT4 * NE], F32, tag="s")
            nc.vector.tensor_tensor(
                s[:].rearrange("p (t e) -> p t e", e=NE), gtv,
                mx[:].unsqueeze(2).broadcast_to([128, NT4, NE]), op=ALU.subtract)
            ex = smallp.tile([128, NT4 * NE], F32, tag="ex")
            nc.scalar.activation(ex[:], s[:], AF.Exp)
            exv = ex[:].rearrange("p (t e) -> p t e", e=NE)
            gt = smallp.tile([128, NT4 * NE * NE], F32, tag="gt")
            a_b = exv.unsqueeze(2).broadcast_to([128, NT4, NE, NE])
            b_b = exv.unsqueeze(3).broadcast_to([128, NT4, NE, NE])
            nc.vector.tensor_tensor(
                gt[:].rearrange("p (t i j) -> p t i j", i=NE, j=NE),
                a_b, b_b, op=ALU.is_gt)
            rank = smallp.tile([128, NT4 * NE], F32, tag="rank")
            nc.vector.reduce_sum(
                rank[:].rearrange("p (t i) -> p t i", i=NE),
                gt[:].rearrange("p (t i j) -> p t i j", i=NE, j=NE), axis=AX.X)
            m01 = smallp.tile([128, NT4 * NE], F32, tag="m01")
            nc.vector.tensor_scalar(m01[:], rank[:], float(TOPK) - 0.5, None,
                                    op0=ALU.is_le)
            wun = smallp.tile([128, NT4 * NE], F32, tag="wun")
            nc.vector.tensor_mul(wun[:], ex[:], m01[:])
            ssum = smallp.tile([128, NT4], F32, tag="ssum")
            nc.vector.reduce_sum(ssum[:], wun[:].rearrange("p (t e) -> p t e", e=NE),
                                 axis=AX.X)
            rinv = smallp.tile([128, NT4], F32, tag="rinv")
            nc.vector.reciprocal(rinv[:], ssum[:])
            wfin = smallp.tile([128, NT4 * NE], F32, tag="wfin")
            nc.vector.tensor_tensor(
                wfin[:].rearrange("p (t e) -> p t e", e=NE),
                wun[:].rearrange("p (t e) -> p t e", e=NE),
                rinv[:].unsqueeze(2).broadcast_to([128, NT4, NE]), op=ALU.mult)

            # ---- head: lep[p, (t e k)] = h2[e][:, t] . smw + smb ----
            lep = hdps.tile([128, NT4 * NE * 10], F32, tag="hd")
            nc.tensor.matmul(lep[:], ones[:], smb[:], start=True, stop=False)
            for t4 in range(NT4):
                tok = slice(128 * t4, 128 * t4 + 128)
                for e in range(NE):
                    cl = slice((t4 * NE + e) * 10, (t4 * NE + e) * 10 + 10)
                    nc.tensor.matmul(lep[:, cl], h2t[e][:, tok], smw[:],
                                     start=False,
                                     stop=(t4 == NT4 - 1 and e == NE - 1))
            scl = smallp.tile([128, NT4 * NE * 10], F32, tag="scl")
            nc.vector.tensor_tensor(
                scl[:].rearrange("p (t e k) -> p t e k", e=NE, k=10),
                lep[:].rearrange("p (t e k) -> p t e k", e=NE, k=10),
                wfin[:].rearrange("p (t e) -> p t e", e=NE)
                    .unsqueeze(3).broadcast_to([128, NT4, NE, 10]),
                op=ALU.mult)
            logit = smallp.tile([128, NT4 * 10], F32, tag="logit")
            nc.vector.reduce_sum(
                logit[:].rearrange("p (t k) -> p t k", k=10),
                scl[:].rearrange("p (t e k) -> p t k e", e=NE, k=10), axis=AX.X)
            lmx = smallp.tile([128, NT4], F32, tag="lmx")
            nc.vector.reduce_max(lmx[:], logit[:].rearrange("p (t k) -> p t k", k=10),
                                 axis=AX.X)
            lsb = smallp.tile([128, NT4 * 10], F32, tag="lsb")
            nc.vector.tensor_tensor(
                lsb[:].rearrange("p (t k) -> p t k", k=10),
                logit[:].rearrange("p (t k) -> p t k", k=10),
                lmx[:].unsqueeze(2).broadcast_to([128, NT4, 10]), op=ALU.subtract)
            lex = smallp.tile([128, NT4 * 10], F32, tag="lex")
            nc.scalar.activation(lex[:], lsb[:], AF.Exp)
            lsum = smallp.tile([128, NT4], F32, tag="lsum")
            nc.vector.reduce_sum(lsum[:], lex[:].rearrange("p (t k) -> p t k", k=10),
                                 axis=AX.X)
            lrinv = smallp.tile([128, NT4], F32, tag="lrinv")
            nc.vector.reciprocal(lrinv[:], lsum[:])
            prob = smallp.tile([128, NT4 * 10], F32, tag="prob")
            nc.vector.tensor_tensor(
                prob[:].rearrange("p (t k) -> p t k", k=10),
                lex[:].rearrange("p (t k) -> p t k", k=10),
                lrinv[:].unsqueeze(2).broadcast_to([128, NT4, 10]), op=ALU.mult)
            dst = out_d[b0:b0 + NB, :].rearrange("(t p) c -> p t c", p=128)
            nc.sync.dma_start(dst, prob[:].rearrange("p (t c) -> p t c", c=10))


_NC_CACHE = None


def _get_nc():
    global _NC_CACHE
    if _NC_CACHE is None:
        _NC_CACHE = _build_nc()
    return _NC_CACHE


def kernel(**inputs):
    xpT, w = _host_prep(**inputs)
    in_maps = []
    for c in range(NCORES):
        m = {"xpT": np.ascontiguousarray(xpT[:, c * BC:(c + 1) * BC])}
        m.update(w)
        in_maps.append(m)
    nc = _get_nc()
    trace = bool(int(os.environ.get("KERNEL_TRACE", "0")))
    res = run_bass_kernel_spmd(nc, in_maps, list(range(NCORES)), trace=trace)
    kernel.last_results = res
    out = np.concatenate([res.results[c]["out"] for c in range(NCORES)], axis=0)
    return out.astype(np.float32)


# revision 12
# speedup vs baseline: 1.2990x; 1.2990x over previous
"""Trainium2 Bass kernel for nn_ClassifyModelMOE (conv feature extractor +
top-3-of-5 MoE + softmax head). Data-parallel over batch across 8 cores.

Self-contained: hardcodes all shapes; builds Toeplitz-expanded conv weights on
the host; runs one SPMD Bass/Tile program on cores 0-7 via run_bass_kernel_spmd.

v2: host-transposed x (plain strided DMAs, no DMA-transpose), SBUF-resident
expert weights (1 startup DMA instead of 250/rep), pool-before-relu in conv1,
batched gating/head per chunk, merged output DMA.
"""
import os
import sys

sys.path.insert(0, "/opt/trn_rl_repo")

import numpy as np
import ml_dtypes

import concourse.bacc as bacc
import concourse.mybir as mybir
import concourse.tile as tile
from concourse.bass_utils import run_bass_kernel_spmd
from concourse.masks import make_identity

F32 = mybir.dt.float32
BF16 = mybir.dt.bfloat16
AF = mybir.ActivationFunctionType
ALU = mybir.AluOpType
AX = mybir.AxisListType

NCORES = 8
B = 8192
BC = B // NCORES          # tokens per core
NB = 512                  # batch chunk (columns per matmul)
NCH = BC // NB            # chunks per core
NE, TOPK = 5, 3
DH = 128
NT4 = NB // 128           # 128-token groups per chunk

# conv1 output geometry: 16ch x 24x24; M-layout (per output row y):
#   Mc0: even x = 2j, j=0..8   -> m = j*16 + o        (128)
#   Mc1: [even j=8..12 | odd j=8..12] -> 64+64        (128)
#   Mc2: odd x = 2j+1, j=0..8  -> m = j*16 + o        (128)
# pooled row tiles: PP0 = j 0..8 (128 parts: j*16+c), PP1 = j 8..12 (64 parts)
# conv2 output (per row y): M = xout*32 + o2:
#   Mc0: xout 0..4 (128), Mc1: xout 4..8 (128),
#   Mc2pair: [y0: xout 8..10 | y1: xout 8..10] (64+64)


def _conv1_cols():
    """(x, o) per (mc, col) for the conv1 M-layout."""
    cols = {0: [], 1: [], 2: []}
    for j in range(8):
        for o in range(16):
            cols[0].append((2 * j, o))
            cols[2].append((2 * j + 1, o))
    for j in range(8, 12):
        for o in range(16):
            cols[1].append((2 * j, o))
    for j in range(8, 12):
        for o in range(16):
            cols[1].append((2 * j + 1, o))
    return cols


def _host_prep(x, conv1_w, conv1_b, conv2_w, conv2_b, gate_w, gate_b,
               e1_w, e1_b, e2_w, e2_b, sm_w, sm_b):
    x = np.asarray(x, np.float32)
    conv1_w = np.asarray(conv1_w, np.float32)
    conv2_w = np.asarray(conv2_w, np.float32)
    gate_w = np.asarray(gate_w, np.float32)
    e1_w = np.asarray(e1_w, np.float32)
    e2_w = np.asarray(e2_w, np.float32)
    bf = ml_dtypes.bfloat16

    # x transposed on host, dense 5-tap K-packing:
    #   xrc  [700, B]: row r*25 + c (c 0..24)  -> serves Mc0/Mc2 (x+dx <= 19)
    #   xrc2 [336, B]: row r*12 + (c-16)       -> serves Mc1 (x 16..23)
    xr = x.reshape(B, 28, 28)
    xrc = np.ascontiguousarray(
        xr[:, :, :25].transpose(1, 2, 0).reshape(700, B)).astype(bf)
    xrc2 = np.ascontiguousarray(
        xr[:, :, 16:28].transpose(1, 2, 0).reshape(336, B)).astype(bf)

    cols = _conv1_cols()
    # w1A [125, 256]: K-row = dy*25 + (x+dx), cols [Mc0 | Mc2]
    # w1B [60, 128]:  K-row = dy*12 + (x+dx-16), cols Mc1
    w1A = np.zeros((125, 256), np.float32)
    w1B = np.zeros((60, 128), np.float32)
    for half, mc in ((0, 0), (1, 2)):
        for ci_col, (xx, o) in enumerate(cols[mc]):
            for dy in range(5):
                for dx in range(5):
                    w1A[dy * 25 + xx + dx, 128 * half + ci_col] = \
                        conv1_w[o, 0, dy, dx]
    for ci_col, (xx, o) in enumerate(cols[1]):
        for dy in range(5):
            for dx in range(5):
                w1B[dy * 12 + xx + dx - 16, ci_col] = conv1_w[o, 0, dy, dx]
    w1A = w1A.astype(bf)
    w1B = w1B.astype(bf)

    # conv2 toeplitz: pooled row layout p = j*16 + c (PP0: j<8), (j-8)*16+c (PP1)
    # w2p0 [3, 128, 256]: r taps, cols [Mc0 | Mc1]
    w2p0 = np.zeros((3, 128, 256), np.float32)
    w2p1mc1 = np.zeros((3, 64, 128), np.float32)
    w2p1mc2 = np.zeros((4, 64, 128), np.float32)
    for r in range(3):
        for j in range(8):
            for c in range(16):
                p = j * 16 + c
                for mci, xobase in ((0, 0), (1, 4)):
                    for xo in range(xobase, xobase + 4):
                        dx = j - xo
                        if 0 <= dx < 3:
                            for o2 in range(32):
                                w2p0[r, p, 128 * mci + (xo - xobase) * 32 + o2] = \
                                    conv2_w[o2, c, r, dx]
        for j in range(8, 12):
            for c in range(16):
                p = (j - 8) * 16 + c
                for xo in range(4, 8):
                    dx = j - xo
                    if 0 <= dx < 3:
                        for o2 in range(32):
                            w2p1mc1[r, p, (xo - 4) * 32 + o2] = conv2_w[o2, c, r, dx]
    for rr in range(4):
        for b_ in range(2):
            r = rr - b_
            if not (0 <= r < 3):
                continue
            for j in range(8, 12):
                for c in range(16):
                    p = (j - 8) * 16 + c
                    for xo in range(8, 10):
                        dx = j - xo
                        if 0 <= dx < 3:
                            for o2 in range(32):
                                w2p1mc2[rr, p, 64 * b_ + (xo - 8) * 32 + o2] = \
                                    conv2_w[o2, c, r, dx]

    # h feature permutation: our flat index (tile*128+part) -> reference f = o2*100 + y*10 + xo
    perm = np.zeros(3200, np.int64)
    for P in range(5):
        y0, y1 = 2 * P, 2 * P + 1
        tiles = []
        for yy in (y0, y1):
            for xobase in (0, 4):
                tiles.append([(yy, xo, o2) for xo in range(xobase, xobase + 4)
                              for o2 in range(32)])
        t4 = [(y0, xo, o2) for xo in range(8, 10) for o2 in range(32)] + \
             [(y1, xo, o2) for xo in range(8, 10) for o2 in range(32)]
        order = [tiles[0], tiles[1], tiles[2], tiles[3], t4]
        for ti, tl in enumerate(order):
            for p, (yy, xo, o2) in enumerate(tl):
                perm[(5 * P + ti) * 128 + p] = o2 * 100 + yy * 10 + xo
    # resident expert-1 weights: [K=128, (e,kc,m)] columns
    e1r = np.ascontiguousarray(
        e1_w[:, perm, :].reshape(NE, 25, 128, DH).transpose(2, 0, 1, 3)
    ).reshape(128, NE * 25 * DH)
    gwp = gate_w[perm, :].reshape(25, 128, NE).astype(np.float32)

    b1col = np.asarray(conv1_b, np.float32)[np.arange(128) % 16].reshape(128, 1)
    b2col = np.asarray(conv2_b, np.float32)[np.arange(128) % 32].reshape(128, 1)
    gbcol = np.asarray(gate_b, np.float32).reshape(NE, 1)
    e1bT = np.asarray(e1_b, np.float32).T.copy()      # [128, 5]
    e2bT = np.asarray(e2_b, np.float32).T.copy()      # [128, 5]
    smw = np.asarray(sm_w, np.float32)                # [128, 10]
    smb = np.tile(np.asarray(sm_b, np.float32), NE * NT4).reshape(1, NE * NT4 * 10)

    # pp1 pair-packing: pp1[2q] in partitions 0..63, pp1[2q+1] in 64..127 of
    # pair[q].  Stacked conv2 weights (rows 0..63 / 64..127 = tap for the
    # lower / upper half):
    #   ev1 = [mc1 r0; mc1 r1]   od = [mc1 r1; mc1 r2]
    #   pA  = [mc2 rr0; mc2 rr1] pB = [mc2 rr2; mc2 rr3]
    #   half = [mc1 r2; mc1 r0]  (K=64 slices at base 0 / base 64)
    w2p1n = np.zeros((128, 640), np.float32)
    w2p1n[0:64, 0:128] = w2p1mc1[0]; w2p1n[64:128, 0:128] = w2p1mc1[1]
    w2p1n[0:64, 128:256] = w2p1mc1[1]; w2p1n[64:128, 128:256] = w2p1mc1[2]
    w2p1n[0:64, 256:384] = w2p1mc2[0]; w2p1n[64:128, 256:384] = w2p1mc2[1]
    w2p1n[0:64, 384:512] = w2p1mc2[2]; w2p1n[64:128, 384:512] = w2p1mc2[3]
    w2p1n[0:64, 512:640] = w2p1mc1[2]; w2p1n[64:128, 512:640] = w2p1mc1[0]

    weights = dict(
        w1A=w1A, w1B=w1B,
        w2p0=np.ascontiguousarray(w2p0.transpose(1, 0, 2)).reshape(128, 768).astype(bf),
        w2p1n=w2p1n.astype(bf),
        e1r=e1r.astype(bf),
        gwp=np.ascontiguousarray(gwp.transpose(1, 0, 2)).reshape(128, 125).astype(bf),
        e2w=np.ascontiguousarray(e2_w.astype(np.float32).transpose(1, 0, 2)).reshape(128, 640).astype(bf),
        b1col=b1col, b2col=b2col,
        gbcol=gbcol, e1bT=e1bT, e2bT=e2bT, smw=smw.astype(bf), smb=smb.astype(bf))
    return xrc, xrc2, weights


def _make_in_maps(inputs):
    xrc, xrc2, w = _host_prep(**inputs)
    in_maps = []
    for c in range(NCORES):
        sl = slice(c * BC, (c + 1) * BC)
        m = {"xrc": np.ascontiguousarray(xrc[:, sl]),
             "xrc2": np.ascontiguousarray(xrc2[:, sl])}
        m.update(w)
        in_maps.append(m)
    return in_maps


def _build_nc(loop_reps=None):
    nc = bacc.Bacc("TRN2", target_bir_lowering=False, debug=False)
    d = {}
    def din(name, shape, dt):
        d[name] = nc.dram_tensor(name, list(shape), dt, kind="ExternalInput").ap()
    din("xrc", (700, BC), BF16)
    din("xrc2", (336, BC), BF16)
    din("w1A", (125, 256), BF16)
    din("w1B", (60, 128), BF16)
    din("w2p0", (128, 768), BF16)
    din("w2p1n", (128, 640), BF16)
    din("e1r", (128, NE * 25 * DH), BF16)
    din("gwp", (128, 125), BF16)
    din("e2w", (128, 640), BF16)
    din("b1col", (128, 1), F32)
    din("b2col", (128, 1), F32)
    din("gbcol", (NE, 1), F32)
    din("e1bT", (128, NE), F32)
    din("e2bT", (128, NE), F32)
    din("smw", (128, 10), BF16)
    din("smb", (1, NE * NT4 * 10), BF16)
    out_d = nc.dram_tensor("out", [BC, 10], F32, kind="ExternalOutput").ap()

    with tile.TileContext(nc) as tc:
        _emit(nc, tc, d, out_d, loop_reps=loop_reps)
    nc.compile()
    return nc


def _emit(nc, tc, d, out_d, loop_reps=None):
    import contextlib
    ctx = contextlib.ExitStack()
    with ctx:
        wpool = ctx.enter_context(tc.tile_pool(name="wpool", bufs=1))
        xtp = ctx.enter_context(tc.tile_pool(name="xtp", bufs=1))
        tmp = ctx.enter_context(tc.tile_pool(name="tmp", bufs=6))
        prp = ctx.enter_context(tc.tile_pool(name="prp", bufs=4))
        shp = ctx.enter_context(tc.tile_pool(name="shp", bufs=2))
        pp0p = ctx.enter_context(tc.tile_pool(name="pp0p", bufs=7))
        pp1p = ctx.enter_context(tc.tile_pool(name="pp1p", bufs=7))
        hpool = ctx.enter_context(tc.tile_pool(name="hpool", bufs=25))
        h1p = ctx.enter_context(tc.tile_pool(name="h1p", bufs=2))
        h2p = ctx.enter_context(tc.tile_pool(name="h2p", bufs=5))
        gp = ctx.enter_context(tc.tile_pool(name="gp", bufs=2))
        smallp = ctx.enter_context(tc.tile_pool(name="smallp", bufs=16))
        c1ps = ctx.enter_context(tc.tile_pool(name="c1ps", bufs=3, space="PSUM"))
        c2ps = ctx.enter_context(tc.tile_pool(name="c2ps", bufs=2, space="PSUM"))
        exps = ctx.enter_context(tc.tile_pool(name="exps", bufs=2, space="PSUM"))
        hdps = ctx.enter_context(tc.tile_pool(name="hdps", bufs=1, space="PSUM"))

        # resident weights
        w1A = wpool.tile([125, 256], BF16); nc.sync.dma_start(w1A[:], d["w1A"][:])
        w1B = wpool.tile([60, 128], BF16); nc.sync.dma_start(w1B[:], d["w1B"][:])
        w2p0 = wpool.tile([128, 3 * 256], BF16)
        nc.sync.dma_start(w2p0[:], d["w2p0"][:])
        w2p1n = wpool.tile([128, 640], BF16)
        nc.sync.dma_start(w2p1n[:], d["w2p1n"][:])
        w2ev1, w2od = w2p1n[:, 0:128], w2p1n[:, 128:256]
        w2pA, w2pB = w2p1n[:, 256:384], w2p1n[:, 384:512]
        w2half = w2p1n[:, 512:640]
        gw = wpool.tile([128, 25 * NE], BF16)
        nc.sync.dma_start(gw[:], d["gwp"][:])
        e2w = wpool.tile([128, NE * DH], BF16)
        nc.sync.dma_start(e2w[:], d["e2w"][:])
        e1w = wpool.tile([128, NE * 25 * DH], BF16)
        for e in range(NE):
            sl = slice(e * 25 * DH, (e + 1) * 25 * DH)
            nc.sync.dma_start(e1w[:, sl], d["e1r"][:, sl])
        b1c = wpool.tile([128, 1], F32); nc.sync.dma_start(b1c[:], d["b1col"][:])
        b2c = wpool.tile([128, 1], F32); nc.sync.dma_start(b2c[:], d["b2col"][:])
        gbc = wpool.tile([NE, 1], F32); nc.sync.dma_start(gbc[:], d["gbcol"][:])
        e1bT = wpool.tile([128, NE], F32); nc.sync.dma_start(e1bT[:], d["e1bT"][:])
        e2bT = wpool.tile([128, NE], F32); nc.sync.dma_start(e2bT[:], d["e2bT"][:])
        smw = wpool.tile([128, 10], BF16); nc.sync.dma_start(smw[:], d["smw"][:])
        smb = wpool.tile([1, NE * NT4 * 10], BF16)
        nc.sync.dma_start(smb[:], d["smb"][:])
        ident = wpool.tile([128, 128], F32)
        make_identity(nc, ident[:])
        ones = wpool.tile([1, 128], BF16)
        nc.scalar.activation(ones[:], e2w[0:1, 0:128], AF.Copy, scale=0.0, bias=1.0)

        import contextlib as _ctl
        loop_cm = tc.For_i(0, loop_reps, 1) if loop_reps else _ctl.nullcontext()
        with loop_cm:
         for ch in range(NCH):
            b0 = ch * NB
            # ---- x tiles: y = 5q+r groups are non-overlapping row windows ----
            gA, gB = [], []
            for r in range(5):
                qn = 5 if r < 4 else 4
                tA = xtp.tile([125, qn * NB], BF16, tag=f"xA{r}")
                src = d["xrc"][25 * r:25 * r + 125 * qn, b0:b0 + NB]
                nc.sync.dma_start(tA[:].rearrange("p (q t) -> p q t", t=NB),
                                  src.rearrange("(q p) t -> p q t", p=125))
                gA.append(tA)
                tB = xtp.tile([60, qn * NB], BF16, tag=f"xB{r}")
                src = d["xrc2"][12 * r:12 * r + 60 * qn, b0:b0 + NB]
                nc.sync.dma_start(tB[:].rearrange("p (q t) -> p q t", t=NB),
                                  src.rearrange("(q p) t -> p q t", p=60))
                gB.append(tB)

            def xA(yy):
                return gA[yy % 5][:, (yy // 5) * NB:(yy // 5 + 1) * NB]

            def xB(yy):
                return gB[yy % 5][:, (yy // 5) * NB:(yy // 5 + 1) * NB]

            # ---- conv1 (single K-dense matmul per (yy, mc)); relu after pool ----
            pp0, pair = [], []
            for Y in range(12):
                y0, y1 = 2 * Y, 2 * Y + 1
                tm = []
                for mc in range(3):
                    pss = []
                    for yy in (y0, y1):
                        ps = c1ps.tile([128, NB], F32, tag="ps")
                        if mc == 0:
                            nc.tensor.matmul(ps[:], w1A[:, 0:128], xA(yy))
                        elif mc == 2:
                            nc.tensor.matmul(ps[:], w1A[:, 128:256], xA(yy))
                        else:
                            nc.tensor.matmul(ps[:], w1B[:], xB(yy))
                        pss.append(ps)
                    # PSUM-PSUM tensor ops are illegal: stage y0 through SBUF
                    c = tmp.tile([128, NB], BF16, tag="cp")
                    nc.scalar.activation(c[:], pss[0][:], AF.Copy)
                    t = tmp.tile([128, NB], BF16, tag="tm")
                    nc.vector.tensor_tensor(t[:], pss[1][:], c[:], op=ALU.max)
                    tm.append(t)
                pre0 = prp.tile([128, NB], BF16, tag="pre0")
                nc.vector.tensor_tensor(pre0[:], tm[0][:], tm[2][:], op=ALU.max)
                p0 = pp0p.tile([128, NB], BF16, tag="pp0")
                nc.vector.tensor_scalar(p0[:], pre0[:], b1c[:, 0:1], 0.0,
                                        op0=ALU.add, op1=ALU.max)
                pp0.append(p0)
                # pp1[Y] -> pair[Y//2], partitions 0..63 (even Y) / 64..127 (odd)
                if Y % 2 == 0:
                    pr = pp1p.tile([128, NB], BF16, tag="pair")
                    pair.append(pr)
                    sh = shp.tile([64, NB], BF16, tag="sh")
                    nc.sync.dma_start(sh[:], tm[1][64:128, :])
                    pre1 = prp.tile([64, NB], BF16, tag="pre1")
                    nc.vector.tensor_tensor(pre1[:], tm[1][0:64, :], sh[:], op=ALU.max)
                    nc.vector.tensor_scalar(pair[-1][0:64, :], pre1[:],
                                            b1c[0:64, 0:1], 0.0,
                                            op0=ALU.add, op1=ALU.max)
                else:
                    sh2 = shp.tile([128, NB], BF16, tag="sh2")
                    nc.sync.dma_start(sh2[64:128, :], tm[1][0:64, :])
                    pre2 = prp.tile([128, NB], BF16, tag="pre2")
                    nc.vector.tensor_tensor(pre2[64:128, :], tm[1][64:128, :],
                                            sh2[64:128, :], op=ALU.max)
                    nc.vector.tensor_scalar(pair[-1][64:128, :], pre2[64:128, :],
                                            b1c[64:128, 0:1], 0.0,
                                            op0=ALU.add, op1=ALU.max)

            # ---- conv2 + relu -> h tiles (pp1 pairs give K=128 streams) ----
            htiles = []
            for P in range(5):
                y0 = 2 * P
                for yy in (y0, y0 + 1):
                    for mci in range(2):
                        ps = c2ps.tile([128, NB], F32, tag="ps")
                        for r in range(3):
                            nc.tensor.matmul(
                                ps[:], w2p0[:, 256 * r + 128 * mci:256 * r + 128 * mci + 128],
                                pp0[yy + r][:], start=(r == 0),
                                stop=(mci == 0 and r == 2))
                        if mci == 1:
                            if yy == y0:
                                nc.tensor.matmul(ps[:], w2ev1, pair[P][:],
                                                 start=False, stop=False)
                                nc.tensor.matmul(ps[:], w2half[0:64, :],
                                                 pair[P + 1][0:64, :],
                                                 start=False, stop=True)
                            else:
                                nc.tensor.matmul(ps[:], w2half[64:128, :],
                                                 pair[P][64:128, :],
                                                 start=False, stop=False)
                                nc.tensor.matmul(ps[:], w2od, pair[P + 1][:],
                                                 start=False, stop=True)
                        h = hpool.tile([128, NB], BF16, tag="h")
                        nc.scalar.activation(h[:], ps[:], AF.Relu, bias=b2c[:, 0:1])
                        htiles.append(h)
                ps = c2ps.tile([128, NB], F32, tag="ps")
                nc.tensor.matmul(ps[:], w2pA, pair[P][:], start=True, stop=False)
                nc.tensor.matmul(ps[:], w2pB, pair[P + 1][:], start=False, stop=True)
                h = hpool.tile([128, NB], BF16, tag="h")
                nc.scalar.activation(h[:], ps[:], AF.Relu, bias=b2c[:, 0:1])
                htiles.append(h)

            # ---- gate ----
            gps = hdps.tile([NE, NB], F32, tag="hd")
            for kc in range(25):
                nc.tensor.matmul(gps[:], gw[:, NE * kc:NE * kc + NE], htiles[kc][:],
                                 start=(kc == 0), stop=(kc == 24))
            gsb = gp.tile([NE, NB], F32, tag="gsb")
            nc.scalar.activation(gsb[:], gps[:], AF.Identity, bias=gbc[:, 0:1])

            # ---- experts ----
            h2t = []
            for e in range(NE):
                h1ps = exps.tile([128, NB], F32, tag="exps")
                for kc in range(25):
                    nc.tensor.matmul(
                        h1ps[:], e1w[:, (e * 25 + kc) * DH:(e * 25 + kc + 1) * DH],
                        htiles[kc][:], start=(kc == 0), stop=(kc == 24))
                h1 = h1p.tile([128, NB], BF16, tag="h1")
                nc.scalar.activation(h1[:], h1ps[:], AF.Tanh, bias=e1bT[:, e:e + 1])
                h2ps = exps.tile([128, NB], F32, tag="exps")
                nc.tensor.matmul(h2ps[:], e2w[:, DH * e:DH * e + DH], h1[:],
                                 start=True, stop=True)
                h2 = h2p.tile([128, NB], BF16, tag="h2")
                nc.scalar.activation(h2[:], h2ps[:], AF.Tanh, bias=e2bT[:, e:e + 1])
                h2t.append(h2)

            # ---- batched gating weights (all 4 t4 groups at once) ----
            gtp = hdps.tile([128, NT4 * NE], F32, tag="hd")
            gtv = gtp[:].rearrange("p (t e) -> p t e", e=NE)
            for t4 in range(NT4):
                tok = slice(128 * t4, 128 * t4 + 128)
                nc.tensor.transpose(gtv[:, t4], gsb[:, tok], ident[0:NE, 0:NE])
            mx = smallp.tile([128, NT4], F32, tag="mx")
            nc.vector.reduce_max(mx[:], gtv, axis=AX.X)
            s = smallp.tile([128, NT4 * NE], F32, tag="s")
            nc.vector.tensor_tensor(
                s[:].rearrange("p (t e) -> p t e", e=NE), gtv,
                mx[:].unsqueeze(2).broadcast_to([128, NT4, NE]), op=ALU.subtract)
            ex = smallp.tile([128, NT4 * NE], F32, tag="ex")
            nc.scalar.activation(ex[:], s[:], AF.Exp)
            exv = ex[:].rearrange("p (t e) -> p t e", e=NE)
            gt = smallp.tile([128, NT4 * NE * NE], F32, tag="gt")
            a_b = exv.unsqueeze(2).broadcast_to([128, NT4, NE, NE])
            b_b = exv.unsqueeze(3).broadcast_to([128, NT4, NE, NE])
            nc.vector.tensor_tensor(
                gt[:].rearrange("p (t i j) -> p t i j", i=NE, j=NE),
                a_b, b_b, op=ALU.is_gt)
            rank = smallp.tile([128, NT4 * NE], F32, tag="rank")
            nc.vector.reduce_sum(
                rank[:].rearrange("p (t i) -> p t i", i=NE),
                gt[:].rearrange("p (t i j) -> p t i j", i=NE, j=NE), axis=AX.X)
            m01 = smallp.tile([128, NT4 * NE], F32, tag="m01")
            nc.vector.tensor_scalar(m01[:], rank[:], float(TOPK) - 0.5, None,
                                    op0=ALU.is_le)
            wun = smallp.tile([128, NT4 * NE], F32, tag="wun")
            nc.vector.tensor_mul(wun[:], ex[:], m01[:])
            ssum = smallp.tile([128, NT4], F32, tag="ssum")
            nc.vector.reduce_sum(ssum[:], wun[:].rearrange("p (t e) -> p t e", e=NE),
                                 axis=AX.X)
            rinv = smallp.tile([128, NT4], F32, tag="rinv")
            nc.vector.reciprocal(rinv[:], ssum[:])
            wfin = smallp.tile([128, NT4 * NE], F32, tag="wfin")
            nc.vector.tensor_tensor(
                wfin[:].rearrange("p (t e) -> p t e", e=NE),
                wun[:].rearrange("p (t e) -> p t e", e=NE),
                rinv[:].unsqueeze(2).broadcast_to([128, NT4, NE]), op=ALU.mult)

            # ---- head: lep[p, (t e k)] = h2[e][:, t] . smw + smb ----
            lep = hdps.tile([128, NT4 * NE * 10], F32, tag="hd")
            nc.tensor.matmul(lep[:], ones[:], smb[:], start=True, stop=False)
            for t4 in range(NT4):
                tok = slice(128 * t4, 128 * t4 + 128)
                for e in range(NE):
                    cl = slice((t4 * NE + e) * 10, (t4 * NE + e) * 10 + 10)
                    nc.tensor.matmul(lep[:, cl], h2t[e][:, tok], smw[:],
                                     start=False,
                                     stop=(t4 == NT4 - 1 and e == NE - 1))
            scl = smallp.tile([128, NT4 * NE * 10], F32, tag="scl")
            nc.vector.tensor_tensor(
                scl[:].rearrange("p (t e k) -> p t e k", e=NE, k=10),
                lep[:].rearrange("p (t e k) -> p t e k", e=NE, k=10),
                wfin[:].rearrange("p (t e) -> p t e", e=NE)
                    .unsqueeze(3).broadcast_to([128, NT4, NE, 10]),
                op=ALU.mult)
            logit = smallp.tile([128, NT4 * 10], F32, tag="logit")
            nc.vector.reduce_sum(
                logit[:].rearrange("p (t k) -> p t k", k=10),
                scl[:].rearrange("p (t e k) -> p t k e", e=NE, k=10), axis=AX.X)
            lmx = smallp.tile([128, NT4], F32, tag="lmx")
            nc.vector.reduce_max(lmx[:], logit[:].rearrange("p (t k) -> p t k", k=10),
                                 axis=AX.X)
            lsb = smallp.tile([128, NT4 * 10], F32, tag="lsb")
            nc.vector.tensor_tensor(
                lsb[:].rearrange("p (t k) -> p t k", k=10),
                logit[:].rearrange("p (t k) -> p t k", k=10),
                lmx[:].unsqueeze(2).broadcast_to([128, NT4, 10]), op=ALU.subtract)
            lex = smallp.tile([128, NT4 * 10], F32, tag="lex")
            nc.scalar.activation(lex[:], lsb[:], AF.Exp)
            lsum = smallp.tile([128, NT4], F32, tag="lsum")
            nc.vector.reduce_sum(lsum[:], lex[:].rearrange("p (t k) -> p t k", k=10),
                                 axis=AX.X)
            lrinv = smallp.tile([128, NT4], F32, tag="lrinv")
            nc.vector.reciprocal(lrinv[:], lsum[:])
            prob = smallp.tile([128, NT4 * 10], F32, tag="prob")
            nc.vector.tensor_tensor(
                prob[:].rearrange("p (t k) -> p t k", k=10),
                lex[:].rearrange("p (t k) -> p t k", k=10),
                lrinv[:].unsqueeze(2).broadcast_to([128, NT4, 10]), op=ALU.mult)
            dst = out_d[b0:b0 + NB, :].rearrange("(t p) c -> p t c", p=128)
            nc.sync.dma_start(dst, prob[:].rearrange("p (t c) -> p t c", c=10))


_NC_CACHE = None


def _get_nc():
    global _NC_CACHE
    if _NC_CACHE is None:
        _NC_CACHE = _build_nc()
    return _NC_CACHE


def kernel(**inputs):
    in_maps = _make_in_maps(inputs)
    nc = _get_nc()
    trace = bool(int(os.environ.get("KERNEL_TRACE", "0")))
    res = run_bass_kernel_spmd(nc, in_maps, list(range(NCORES)), trace=trace)
    kernel.last_results = res
    out = np.concatenate([res.results[c]["out"] for c in range(NCORES)], axis=0)
    return out.astype(np.float32)


# revision 17
# speedup vs baseline: 1.3248x; 1.0199x over previous
"""Trainium2 Bass kernel for nn_ClassifyModelMOE (conv feature extractor +
top-3-of-5 MoE + softmax head). Data-parallel over batch across 8 cores.

Self-contained: hardcodes all shapes; builds Toeplitz-expanded conv weights on
the host; runs one SPMD Bass/Tile program on cores 0-7 via run_bass_kernel_spmd.

v2: host-transposed x (plain strided DMAs, no DMA-transpose), SBUF-resident
expert weights (1 startup DMA instead of 250/rep), pool-before-relu in conv1,
batched gating/head per chunk, merged output DMA.
"""
import os
import sys

sys.path.insert(0, "/opt/trn_rl_repo")

import numpy as np
import ml_dtypes

import concourse.bacc as bacc
import concourse.mybir as mybir
import concourse.tile as tile
from concourse.bass_utils import run_bass_kernel_spmd
from concourse.masks import make_identity

F32 = mybir.dt.float32
BF16 = mybir.dt.bfloat16
F8 = mybir.dt.float8e4
DR = mybir.MatmulPerfMode.DoubleRow
WS_INV = 1.0 / 512.0
AF = mybir.ActivationFunctionType
ALU = mybir.AluOpType
AX = mybir.AxisListType

NCORES = 8
B = 8192
BC = B // NCORES          # tokens per core
NB = 512                  # batch chunk (columns per matmul)
NCH = BC // NB            # chunks per core
NE, TOPK = 5, 3
DH = 128
NT4 = NB // 128           # 128-token groups per chunk

# conv1 output geometry: 16ch x 24x24; M-layout (per output row y):
#   Mc0: even x = 2j, j=0..8   -> m = j*16 + o        (128)
#   Mc1: [even j=8..12 | odd j=8..12] -> 64+64        (128)
#   Mc2: odd x = 2j+1, j=0..8  -> m = j*16 + o        (128)
# pooled row tiles: PP0 = j 0..8 (128 parts: j*16+c), PP1 = j 8..12 (64 parts)
# conv2 output (per row y): M = xout*32 + o2:
#   Mc0: xout 0..4 (128), Mc1: xout 4..8 (128),
#   Mc2pair: [y0: xout 8..10 | y1: xout 8..10] (64+64)


def _conv1_cols():
    """(x, o) per (mc, col) for the conv1 M-layout."""
    cols = {0: [], 1: [], 2: []}
    for j in range(8):
        for o in range(16):
            cols[0].append((2 * j, o))
            cols[2].append((2 * j + 1, o))
    for j in range(8, 12):
        for o in range(16):
            cols[1].append((2 * j, o))
    for j in range(8, 12):
        for o in range(16):
            cols[1].append((2 * j + 1, o))
    return cols


def _host_prep(x, conv1_w, conv1_b, conv2_w, conv2_b, gate_w, gate_b,
               e1_w, e1_b, e2_w, e2_b, sm_w, sm_b):
    x = np.asarray(x, np.float32)
    conv1_w = np.asarray(conv1_w, np.float32)
    conv2_w = np.asarray(conv2_w, np.float32)
    gate_w = np.asarray(gate_w, np.float32)
    e1_w = np.asarray(e1_w, np.float32)
    e2_w = np.asarray(e2_w, np.float32)
    bf = ml_dtypes.bfloat16

    # x transposed on host, dense 5-tap K-packing:
    #   xrc  [700, B]: row r*25 + c (c 0..24)  -> serves Mc0/Mc2 (x+dx <= 19)
    #   xrc2 [336, B]: row r*12 + (c-16)       -> serves Mc1 (x 16..23)
    xr = x.reshape(B, 28, 28)
    xrc = np.ascontiguousarray(
        xr[:, :, :25].transpose(1, 2, 0).reshape(700, B)).astype(bf)
    xrc2 = np.ascontiguousarray(
        xr[:, :, 16:28].transpose(1, 2, 0).reshape(336, B)).astype(bf)

    cols = _conv1_cols()
    # w1A [125, 256]: K-row = dy*25 + (x+dx), cols [Mc0 | Mc2]
    # w1B [60, 128]:  K-row = dy*12 + (x+dx-16), cols Mc1
    w1A = np.zeros((125, 256), np.float32)
    w1B = np.zeros((60, 128), np.float32)
    for half, mc in ((0, 0), (1, 2)):
        for ci_col, (xx, o) in enumerate(cols[mc]):
            for dy in range(5):
                for dx in range(5):
                    w1A[dy * 25 + xx + dx, 128 * half + ci_col] = \
                        conv1_w[o, 0, dy, dx]
    for ci_col, (xx, o) in enumerate(cols[1]):
        for dy in range(5):
            for dx in range(5):
                w1B[dy * 12 + xx + dx - 16, ci_col] = conv1_w[o, 0, dy, dx]
    w1A = w1A.astype(bf)
    w1B = w1B.astype(bf)

    # conv2 toeplitz: pooled row layout p = j*16 + c (PP0: j<8), (j-8)*16+c (PP1)
    # w2p0 [3, 128, 256]: r taps, cols [Mc0 | Mc1]
    w2p0 = np.zeros((3, 128, 256), np.float32)
    w2p1mc1 = np.zeros((3, 64, 128), np.float32)
    w2p1mc2 = np.zeros((4, 64, 128), np.float32)
    for r in range(3):
        for j in range(8):
            for c in range(16):
                p = j * 16 + c
                for mci, xobase in ((0, 0), (1, 4)):
                    for xo in range(xobase, xobase + 4):
                        dx = j - xo
                        if 0 <= dx < 3:
                            for o2 in range(32):
                                w2p0[r, p, 128 * mci + (xo - xobase) * 32 + o2] = \
                                    conv2_w[o2, c, r, dx]
        for j in range(8, 12):
            for c in range(16):
                p = (j - 8) * 16 + c
                for xo in range(4, 8):
                    dx = j - xo
                    if 0 <= dx < 3:
                        for o2 in range(32):
                            w2p1mc1[r, p, (xo - 4) * 32 + o2] = conv2_w[o2, c, r, dx]
    for rr in range(4):
        for b_ in range(2):
            r = rr - b_
            if not (0 <= r < 3):
                continue
            for j in range(8, 12):
                for c in range(16):
                    p = (j - 8) * 16 + c
                    for xo in range(8, 10):
                        dx = j - xo
                        if 0 <= dx < 3:
                            for o2 in range(32):
                                w2p1mc2[rr, p, 64 * b_ + (xo - 8) * 32 + o2] = \
                                    conv2_w[o2, c, r, dx]

    # h feature permutation: our flat index (tile*128+part) -> reference f = o2*100 + y*10 + xo
    perm = np.zeros(3200, np.int64)
    for P in range(5):
        y0, y1 = 2 * P, 2 * P + 1
        tiles = []
        for yy in (y0, y1):
            for xobase in (0, 4):
                tiles.append([(yy, xo, o2) for xo in range(xobase, xobase + 4)
                              for o2 in range(32)])
        t4 = [(y0, xo, o2) for xo in range(8, 10) for o2 in range(32)] + \
             [(y1, xo, o2) for xo in range(8, 10) for o2 in range(32)]
        order = [tiles[0], tiles[1], tiles[2], tiles[3], t4]
        for ti, tl in enumerate(order):
            for p, (yy, xo, o2) in enumerate(tl):
                perm[(5 * P + ti) * 128 + p] = o2 * 100 + yy * 10 + xo
    # resident expert-1 / gate weights in fp8e4m3, scaled by 512 (raw values
    # ~N(0, 0.018) are subnormal in e4m3); compensated by scale=1/512 on the
    # PSUM-evicting activation.  DoubleRow layout: 12 kc-pairs [K=128, ks=2, M]
    # + kc24 single, flattened to columns e*step + a*2M + ks*M + m.
    WS = 512.0
    f8 = ml_dtypes.float8_e4m3
    e1p = e1_w[:, perm, :].reshape(NE, 25, 128, DH).transpose(2, 0, 1, 3) * WS
    # [128, NE, 25, DH] -> per e: pairs a: cols [2a, 2a+1] interleaved
    e1q = np.zeros((128, NE * 25 * DH), np.float32)
    for e in range(NE):
        base = e * 25 * DH
        for a in range(12):
            blk = e1p[:, e, 2 * a:2 * a + 2, :]          # [128, 2, DH]
            e1q[:, base + 2 * a * DH:base + (2 * a + 2) * DH] = \
                blk.reshape(128, 2 * DH)
        e1q[:, base + 24 * DH:base + 25 * DH] = e1p[:, e, 24, :]
    gp8 = gate_w[perm, :].reshape(25, 128, NE).transpose(1, 0, 2) * WS  # [128,25,5]
    gwq = np.ascontiguousarray(gp8).reshape(128, 125)

    b1col = np.asarray(conv1_b, np.float32)[np.arange(128) % 16].reshape(128, 1)
    b2col = np.asarray(conv2_b, np.float32)[np.arange(128) % 32].reshape(128, 1)
    gbcol = np.asarray(gate_b, np.float32).reshape(NE, 1)
    e1bT = np.asarray(e1_b, np.float32).T.copy()      # [128, 5]
    e2bT = np.asarray(e2_b, np.float32).T.copy()      # [128, 5]
    smw = np.asarray(sm_w, np.float32)                # [128, 10]
    smb = np.tile(np.asarray(sm_b, np.float32), NE * NT4).reshape(1, NE * NT4 * 10)

    # pp1 pair-packing: pp1[2q] in partitions 0..63, pp1[2q+1] in 64..127 of
    # pair[q].  Stacked conv2 weights (rows 0..63 / 64..127 = tap for the
    # lower / upper half):
    #   ev1 = [mc1 r0; mc1 r1]   od = [mc1 r1; mc1 r2]
    #   pA  = [mc2 rr0; mc2 rr1] pB = [mc2 rr2; mc2 rr3]
    #   half = [mc1 r2; mc1 r0]  (K=64 slices at base 0 / base 64)
    w2p1n = np.zeros((128, 640), np.float32)
    w2p1n[0:64, 0:128] = w2p1mc1[0]; w2p1n[64:128, 0:128] = w2p1mc1[1]
    w2p1n[0:64, 128:256] = w2p1mc1[1]; w2p1n[64:128, 128:256] = w2p1mc1[2]
    w2p1n[0:64, 256:384] = w2p1mc2[0]; w2p1n[64:128, 256:384] = w2p1mc2[1]
    w2p1n[0:64, 384:512] = w2p1mc2[2]; w2p1n[64:128, 384:512] = w2p1mc2[3]
    w2p1n[0:64, 512:640] = w2p1mc1[2]; w2p1n[64:128, 512:640] = w2p1mc1[0]

    weights = dict(
        w1A=w1A, w1B=w1B,
        w2p0=np.ascontiguousarray(w2p0.transpose(1, 0, 2)).reshape(128, 768).astype(bf),
        w2p1n=w2p1n.astype(bf),
        e1r=e1q.astype(f8),
        gwp=gwq.astype(f8),
        e2w=np.ascontiguousarray(e2_w.astype(np.float32).transpose(1, 0, 2)).reshape(128, 640).astype(bf),
        b1col=b1col, b2col=b2col,
        gbcol=gbcol, e1bT=e1bT, e2bT=e2bT, smw=smw.astype(bf), smb=smb.astype(bf))
    return xrc, xrc2, weights


def _make_in_maps(inputs):
    xrc, xrc2, w = _host_prep(**inputs)
    in_maps = []
    for c in range(NCORES):
        sl = slice(c * BC, (c + 1) * BC)
        m = {"xrc": np.ascontiguousarray(xrc[:, sl]),
             "xrc2": np.ascontiguousarray(xrc2[:, sl])}
        m.update(w)
        in_maps.append(m)
    return in_maps


def _build_nc(loop_reps=None):
    nc = bacc.Bacc("TRN2", target_bir_lowering=False, debug=False)
    d = {}
    def din(name, shape, dt):
        d[name] = nc.dram_tensor(name, list(shape), dt, kind="ExternalInput").ap()
    din("xrc", (700, BC), BF16)
    din("xrc2", (336, BC), BF16)
    din("w1A", (125, 256), BF16)
    din("w1B", (60, 128), BF16)
    din("w2p0", (128, 768), BF16)
    din("w2p1n", (128, 640), BF16)
    din("e1r", (128, NE * 25 * DH), F8)
    din("gwp", (128, 125), F8)
    din("e2w", (128, 640), BF16)
    din("b1col", (128, 1), F32)
    din("b2col", (128, 1), F32)
    din("gbcol", (NE, 1), F32)
    din("e1bT", (128, NE), F32)
    din("e2bT", (128, NE), F32)
    din("smw", (128, 10), BF16)
    din("smb", (1, NE * NT4 * 10), BF16)
    out_d = nc.dram_tensor("out", [BC, 10], F32, kind="ExternalOutput").ap()

    with tile.TileContext(nc) as tc:
        _emit(nc, tc, d, out_d, loop_reps=loop_reps)
    nc.compile()
    return nc


def _emit(nc, tc, d, out_d, loop_reps=None):
    import contextlib
    ctx = contextlib.ExitStack()
    with ctx:
        wpool = ctx.enter_context(tc.tile_pool(name="wpool", bufs=1))
        xtp = ctx.enter_context(tc.tile_pool(name="xtp", bufs=1))
        tmp = ctx.enter_context(tc.tile_pool(name="tmp", bufs=6))
        prp = ctx.enter_context(tc.tile_pool(name="prp", bufs=4))
        shp = ctx.enter_context(tc.tile_pool(name="shp", bufs=2))
        pp0p = ctx.enter_context(tc.tile_pool(name="pp0p", bufs=7))
        pp1p = ctx.enter_context(tc.tile_pool(name="pp1p", bufs=7))
        hpool = ctx.enter_context(tc.tile_pool(name="hpool", bufs=1))
        h1p = ctx.enter_context(tc.tile_pool(name="h1p", bufs=2))
        h2p = ctx.enter_context(tc.tile_pool(name="h2p", bufs=5))
        gp = ctx.enter_context(tc.tile_pool(name="gp", bufs=2))
        smallp = ctx.enter_context(tc.tile_pool(name="smallp", bufs=16))
        c1ps = ctx.enter_context(tc.tile_pool(name="c1ps", bufs=3, space="PSUM"))
        c2ps = ctx.enter_context(tc.tile_pool(name="c2ps", bufs=2, space="PSUM"))
        exps = ctx.enter_context(tc.tile_pool(name="exps", bufs=2, space="PSUM"))
        hdps = ctx.enter_context(tc.tile_pool(name="hdps", bufs=1, space="PSUM"))

        # resident weights
        w1A = wpool.tile([125, 256], BF16); nc.sync.dma_start(w1A[:], d["w1A"][:])
        w1B = wpool.tile([60, 128], BF16); nc.sync.dma_start(w1B[:], d["w1B"][:])
        w2p0 = wpool.tile([128, 3 * 256], BF16)
        nc.sync.dma_start(w2p0[:], d["w2p0"][:])
        w2p1n = wpool.tile([128, 640], BF16)
        nc.sync.dma_start(w2p1n[:], d["w2p1n"][:])
        w2ev1, w2od = w2p1n[:, 0:128], w2p1n[:, 128:256]
        w2pA, w2pB = w2p1n[:, 256:384], w2p1n[:, 384:512]
        w2half = w2p1n[:, 512:640]
        gw = wpool.tile([128, 25 * NE], F8)
        nc.sync.dma_start(gw[:], d["gwp"][:])
        e2w = wpool.tile([128, NE * DH], BF16)
        nc.sync.dma_start(e2w[:], d["e2w"][:])
        e1w = wpool.tile([128, NE * 25 * DH], F8)
        for e in range(NE):
            sl = slice(e * 25 * DH, (e + 1) * 25 * DH)
            nc.sync.dma_start(e1w[:, sl], d["e1r"][:, sl])
        b1c = wpool.tile([128, 1], F32); nc.sync.dma_start(b1c[:], d["b1col"][:])
        b2c = wpool.tile([128, 1], F32); nc.sync.dma_start(b2c[:], d["b2col"][:])
        gbc = wpool.tile([NE, 1], F32); nc.sync.dma_start(gbc[:], d["gbcol"][:])
        e1bT = wpool.tile([128, NE], F32); nc.sync.dma_start(e1bT[:], d["e1bT"][:])
        e2bT = wpool.tile([128, NE], F32); nc.sync.dma_start(e2bT[:], d["e2bT"][:])
        smw = wpool.tile([128, 10], BF16); nc.sync.dma_start(smw[:], d["smw"][:])
        smb = wpool.tile([1, NE * NT4 * 10], BF16)
        nc.sync.dma_start(smb[:], d["smb"][:])
        ident = wpool.tile([128, 128], F32)
        make_identity(nc, ident[:])
        ones = wpool.tile([1, 128], BF16)
        nc.scalar.activation(ones[:], e2w[0:1, 0:128], AF.Copy, scale=0.0, bias=1.0)

        import contextlib as _ctl
        loop_cm = tc.For_i(0, loop_reps, 1) if loop_reps else _ctl.nullcontext()
        with loop_cm:
         for ch in range(NCH):
            b0 = ch * NB
            # ---- x tiles: y = 5q+r groups are non-overlapping row windows ----
            gA, gB = [], []
            for r in range(5):
                qn = 5 if r < 4 else 4
                tA = xtp.tile([125, qn * NB], BF16, tag=f"xA{r}")
                src = d["xrc"][25 * r:25 * r + 125 * qn, b0:b0 + NB]
                nc.sync.dma_start(tA[:].rearrange("p (q t) -> p q t", t=NB),
                                  src.rearrange("(q p) t -> p q t", p=125))
                gA.append(tA)
                tB = xtp.tile([60, qn * NB], BF16, tag=f"xB{r}")
                src = d["xrc2"][12 * r:12 * r + 60 * qn, b0:b0 + NB]
                nc.sync.dma_start(tB[:].rearrange("p (q t) -> p q t", t=NB),
                                  src.rearrange("(q p) t -> p q t", p=60))
                gB.append(tB)

            def xA(yy):
                return gA[yy % 5][:, (yy // 5) * NB:(yy // 5 + 1) * NB]

            def xB(yy):
                return gB[yy % 5][:, (yy // 5) * NB:(yy // 5 + 1) * NB]

            # ---- conv1 (single K-dense matmul per (yy, mc)); relu after pool ----
            pp0, pair = [], []
            for Y in range(12):
                y0, y1 = 2 * Y, 2 * Y + 1
                tm = []
                for mc in range(3):
                    pss = []
                    for yy in (y0, y1):
                        ps = c1ps.tile([128, NB], F32, tag="ps")
                        if mc == 0:
                            nc.tensor.matmul(ps[:], w1A[:, 0:128], xA(yy))
                        elif mc == 2:
                            nc.tensor.matmul(ps[:], w1A[:, 128:256], xA(yy))
                        else:
                            nc.tensor.matmul(ps[:], w1B[:], xB(yy))
                        pss.append(ps)
                    # PSUM-PSUM tensor ops are illegal: stage y0 through SBUF
                    c = tmp.tile([128, NB], BF16, tag="cp")
                    nc.scalar.activation(c[:], pss[0][:], AF.Copy)
                    t = tmp.tile([128, NB], BF16, tag="tm")
                    nc.vector.tensor_tensor(t[:], pss[1][:], c[:], op=ALU.max)
                    tm.append(t)
                pre0 = prp.tile([128, NB], BF16, tag="pre0")
                nc.vector.tensor_tensor(pre0[:], tm[0][:], tm[2][:], op=ALU.max)
                p0 = pp0p.tile([128, NB], BF16, tag="pp0")
                nc.vector.tensor_scalar(p0[:], pre0[:], b1c[:, 0:1], 0.0,
                                        op0=ALU.add, op1=ALU.max)
                pp0.append(p0)
                # pp1[Y] -> pair[Y//2], partitions 0..63 (even Y) / 64..127 (odd)
                if Y % 2 == 0:
                    pr = pp1p.tile([128, NB], BF16, tag="pair")
                    pair.append(pr)
                    sh = shp.tile([64, NB], BF16, tag="sh")
                    nc.sync.dma_start(sh[:], tm[1][64:128, :])
                    pre1 = prp.tile([64, NB], BF16, tag="pre1")
                    nc.vector.tensor_tensor(pre1[:], tm[1][0:64, :], sh[:], op=ALU.max)
                    nc.vector.tensor_scalar(pair[-1][0:64, :], pre1[:],
                                            b1c[0:64, 0:1], 0.0,
                                            op0=ALU.add, op1=ALU.max)
                else:
                    sh2 = shp.tile([128, NB], BF16, tag="sh2")
                    nc.sync.dma_start(sh2[64:128, :], tm[1][0:64, :])
                    pre2 = prp.tile([128, NB], BF16, tag="pre2")
                    nc.vector.tensor_tensor(pre2[64:128, :], tm[1][64:128, :],
                                            sh2[64:128, :], op=ALU.max)
                    nc.vector.tensor_scalar(pair[-1][64:128, :], pre2[64:128, :],
                                            b1c[64:128, 0:1], 0.0,
                                            op0=ALU.add, op1=ALU.max)

            # ---- conv2 + relu -> h tiles (pp1 pairs give K=128 streams) ----
            hbig = hpool.tile([128, 25 * NB], F8, tag="h")
            htiles = [hbig[:, kc * NB:(kc + 1) * NB] for kc in range(25)]
            hi = 0
            for P in range(5):
                y0 = 2 * P
                for yy in (y0, y0 + 1):
                    for mci in range(2):
                        ps = c2ps.tile([128, NB], F32, tag="ps")
                        for r in range(3):
                            nc.tensor.matmul(
                                ps[:], w2p0[:, 256 * r + 128 * mci:256 * r + 128 * mci + 128],
                                pp0[yy + r][:], start=(r == 0),
                                stop=(mci == 0 and r == 2))
                        if mci == 1:
                            if yy == y0:
                                nc.tensor.matmul(ps[:], w2ev1, pair[P][:],
                                                 start=False, stop=False)
                                nc.tensor.matmul(ps[:], w2half[0:64, :],
                                                 pair[P + 1][0:64, :],
                                                 start=False, stop=True)
                            else:
                                nc.tensor.matmul(ps[:], w2half[64:128, :],
                                                 pair[P][64:128, :],
                                                 start=False, stop=False)
                                nc.tensor.matmul(ps[:], w2od, pair[P + 1][:],
                                                 start=False, stop=True)
                        nc.scalar.activation(htiles[hi], ps[:], AF.Relu,
                                             bias=b2c[:, 0:1])
                        hi += 1
                ps = c2ps.tile([128, NB], F32, tag="ps")
                nc.tensor.matmul(ps[:], w2pA, pair[P][:], start=True, stop=False)
                nc.tensor.matmul(ps[:], w2pB, pair[P + 1][:], start=False, stop=True)
                nc.scalar.activation(htiles[hi], ps[:], AF.Relu, bias=b2c[:, 0:1])
                hi += 1

            # ---- gate ----
            gps = hdps.tile([NE, NB], F32, tag="hd")
            for kc in range(25):
                nc.tensor.matmul(gps[:], gw[:, NE * kc:NE * kc + NE], htiles[kc],
                                 start=(kc == 0), stop=(kc == 24))
            gsb = gp.tile([NE, NB], F32, tag="gsb")
            nc.scalar.activation(gsb[:], gps[:], AF.Identity, scale=WS_INV,
                                 bias=gbc[:, 0:1])

            # ---- experts ----
            h2t = []
            for e in range(NE):
                h1ps = exps.tile([128, NB], F32, tag="exps")
                base = e * 25 * DH
                for a in range(12):
                    nc.tensor.matmul(
                        h1ps[:],
                        e1w[:, base + 2 * a * DH:base + (2 * a + 2) * DH]
                            .rearrange("p (ks m) -> p ks m", ks=2),
                        hbig[:, 2 * a * NB:(2 * a + 2) * NB]
                            .rearrange("p (ks n) -> p ks n", ks=2),
                        start=(a == 0), stop=False, perf_mode=DR)
                nc.tensor.matmul(h1ps[:], e1w[:, base + 24 * DH:base + 25 * DH],
                                 htiles[24], start=False, stop=True)
                h1 = h1p.tile([128, NB], BF16, tag="h1")
                nc.scalar.activation(h1[:], h1ps[:], AF.Tanh, scale=WS_INV,
                                     bias=e1bT[:, e:e + 1])
                h2ps = exps.tile([128, NB], F32, tag="exps")
                nc.tensor.matmul(h2ps[:], e2w[:, DH * e:DH * e + DH], h1[:],
                                 start=True, stop=True)
                h2 = h2p.tile([128, NB], BF16, tag="h2")
                nc.scalar.activation(h2[:], h2ps[:], AF.Tanh, bias=e2bT[:, e:e + 1])
                h2t.append(h2)

            # ---- batched gating weights (all 4 t4 groups at once) ----
            gtp = hdps.tile([128, NT4 * NE], F32, tag="hd")
            gtv = gtp[:].rearrange("p (t e) -> p t e", e=NE)
            for t4 in range(NT4):
                tok = slice(128 * t4, 128 * t4 + 128)
                nc.tensor.transpose(gtv[:, t4], gsb[:, tok], ident[0:NE, 0:NE])
            mx = smallp.tile([128, NT4], F32, tag="mx")
            nc.vector.reduce_max(mx[:], gtv, axis=AX.X)
            s = smallp.tile([128, NT4 * NE], F32, tag="s")
            nc.vector.tensor_tensor(
                s[:].rearrange("p (t e) -> p t e", e=NE), gtv,
                mx[:].unsqueeze(2).broadcast_to([128, NT4, NE]), op=ALU.subtract)
            ex = smallp.tile([128, NT4 * NE], F32, tag="ex")
            nc.scalar.activation(ex[:], s[:], AF.Exp)
            exv = ex[:].rearrange("p (t e) -> p t e", e=NE)
            gt = smallp.tile([128, NT4 * NE * NE], F32, tag="gt")
            a_b = exv.unsqueeze(2).broadcast_to([128, NT4, NE, NE])
            b_b = exv.unsqueeze(3).broadcast_to([128, NT4, NE, NE])
            nc.vector.tensor_tensor(
                gt[:].rearrange("p (t i j) -> p t i j", i=NE, j=NE),
                a_b, b_b, op=ALU.is_gt)
            rank = smallp.tile([128, NT4 * NE], F32, tag="rank")
            nc.vector.reduce_sum(
                rank[:].rearrange("p (t i) -> p t i", i=NE),
                gt[:].rearrange("p (t i j) -> p t i j", i=NE, j=NE), axis=AX.X)
            m01 = smallp.tile([128, NT4 * NE], F32, tag="m01")
            nc.vector.tensor_scalar(m01[:], rank[:], float(TOPK) - 0.5, None,
                                    op0=ALU.is_le)
            wun = smallp.tile([128, NT4 * NE], F32, tag="wun")
            nc.vector.tensor_mul(wun[:], ex[:], m01[:])
            ssum = smallp.tile([128, NT4], F32, tag="ssum")
            nc.vector.reduce_sum(ssum[:], wun[:].rearrange("p (t e) -> p t e", e=NE),
                                 axis=AX.X)
            rinv = smallp.tile([128, NT4], F32, tag="rinv")
            nc.vector.reciprocal(rinv[:], ssum[:])
            wfin = smallp.tile([128, NT4 * NE], F32, tag="wfin")
            nc.vector.tensor_tensor(
                wfin[:].rearrange("p (t e) -> p t e", e=NE),
                wun[:].rearrange("p (t e) -> p t e", e=NE),
                rinv[:].unsqueeze(2).broadcast_to([128, NT4, NE]), op=ALU.mult)

            # ---- head: lep[p, (t e k)] = h2[e][:, t] . smw + smb ----
            lep = hdps.tile([128, NT4 * NE * 10], F32, tag="hd")
            nc.tensor.matmul(lep[:], ones[:], smb[:], start=True, stop=False)
            for t4 in range(NT4):
                tok = slice(128 * t4, 128 * t4 + 128)
                for e in range(NE):
                    cl = slice((t4 * NE + e) * 10, (t4 * NE + e) * 10 + 10)
                    nc.tensor.matmul(lep[:, cl], h2t[e][:, tok], smw[:],
                                     start=False,
                                     stop=(t4 == NT4 - 1 and e == NE - 1))
            scl = smallp.tile([128, NT4 * NE * 10], F32, tag="scl")
            nc.vector.tensor_tensor(
                scl[:].rearrange("p (t e k) -> p t e k", e=NE, k=10),
                lep[:].rearrange("p (t e k) -> p t e k", e=NE, k=10),
                wfin[:].rearrange("p (t e) -> p t e", e=NE)
                    .unsqueeze(3).broadcast_to([128, NT4, NE, 10]),
                op=ALU.mult)
            logit = smallp.tile([128, NT4 * 10], F32, tag="logit")
            nc.vector.reduce_sum(
                logit[:].rearrange("p (t k) -> p t k", k=10),
                scl[:].rearrange("p (t e k) -> p t k e", e=NE, k=10), axis=AX.X)
            lmx = smallp.tile([128, NT4], F32, tag="lmx")
            nc.vector.reduce_max(lmx[:], logit[:].rearrange("p (t k) -> p t k", k=10),
                                 axis=AX.X)
            lsb = smallp.tile([128, NT4 * 10], F32, tag="lsb")
            nc.vector.tensor_tensor(
                lsb[:].rearrange("p (t k) -> p t k", k=10),
                logit[:].rearrange("p (t k) -> p t k", k=10),
                lmx[:].unsqueeze(2).broadcast_to([128, NT4, 10]), op=ALU.subtract)
            lex = smallp.tile([128, NT4 * 10], F32, tag="lex")
            nc.scalar.activation(lex[:], lsb[:], AF.Exp)
            lsum = smallp.tile([128, NT4], F32, tag="lsum")
            nc.vector.reduce_sum(lsum[:], lex[:].rearrange("p (t k) -> p t k", k=10),
                                 axis=AX.X)
            lrinv = smallp.tile([128, NT4], F32, tag="lrinv")
            nc.vector.reciprocal(lrinv[:], lsum[:])
            prob = smallp.tile([128, NT4 * 10], F32, tag="prob")
            nc.vector.tensor_tensor(
                prob[:].rearrange("p (t k) -> p t k", k=10),
                lex[:].rearrange("p (t k) -> p t k", k=10),
                lrinv[:].unsqueeze(2).broadcast_to([128, NT4, 10]), op=ALU.mult)
            dst = out_d[b0:b0 + NB, :].rearrange("(t p) c -> p t c", p=128)
            nc.sync.dma_start(dst, prob[:].rearrange("p (t c) -> p t c", c=10))


_NC_CACHE = None


def _get_nc():
    global _NC_CACHE
    if _NC_CACHE is None:
        _NC_CACHE = _build_nc()
    return _NC_CACHE


def kernel(**inputs):
    in_maps = _make_in_maps(inputs)
    nc = _get_nc()
    trace = bool(int(os.environ.get("KERNEL_TRACE", "0")))
    res = run_bass_kernel_spmd(nc, in_maps, list(range(NCORES)), trace=trace)
    kernel.last_results = res
    out = np.concatenate([res.results[c]["out"] for c in range(NCORES)], axis=0)
    return out.astype(np.float32)
